# revision 5
# baseline (speedup 1.0000x reference)
"""Dilated (segment-local) attention kernel for Trainium2, 8 NeuronCores.

Reference computation (all shapes hardcoded):
    x: (4, 8192, 1024) f32
    segments of 1024 tokens, dilation 2 -> 32 independent blocks of
    (512 tokens, 1024 dim); self-attention (q=k=v=x) within each block;
    output (4, 4096, 1024) f32.

Sharding: the 32 blocks are fully independent -> 4 blocks per core.

Device algorithm per block (all matmuls bf16, fp32 PSUM accumulate):
    S = Xd @ Xd^T            (Xd = dilated block, scores before 1/sqrt(D))
    E = exp(S/32)            unnormalized probs. S is symmetric (q=k), so E
                             is symmetric: E row-chunks are reusable as the
                             transposed stationary operand of the second
                             matmul -- no on-chip transposes needed.
    row_sums come free via the activation's accum_out.
    O = (E @ V) * (1/row_sums)   normalization fused into PSUM eviction.

No max-subtraction: S/32 has diagonal ~32 +- 2 and off-diagonal ~N(0,1),
so exp() stays far below fp32 overflow and the reference softmax (which
does subtract the max) is matched to bf16-input accuracy.
"""

import numpy as np
import ml_dtypes

import concourse.bass as bass
import concourse.bacc as bacc
import concourse.tile as tile
from concourse import mybir
from concourse.bass_utils import run_bass_kernel_spmd

BF16 = mybir.dt.bfloat16
F32 = mybir.dt.float32

N_CORES = 8
B, S, D = 4, 8192, 1024
SEG = 1024
DIL = 2
TOK = SEG // DIL          # 512 tokens per block after dilation
NSEG = S // SEG           # 8
NBLK = B * NSEG           # 32
BPC = NBLK // N_CORES     # 4 blocks per core
TC = TOK // 128           # 4 token chunks of 128
DC = D // 128             # 8 dim chunks of 128
NH = D // 512             # 2 output halves (psum bank = 512 f32)
SCALE = 1.0 / 32.0        # 1/sqrt(D)


def build_bass() -> bass.Bass:
    # Bacc (not raw Bass): its compile() runs generate_event_semaphores,
    # which splits multi-sem waits (TRN2 allows 1 wait per instruction).
    nc = bacc.Bacc()
    xt = nc.declare_dram_parameter("xt", [BPC, D, TOK], BF16, isOutput=False)
    v = nc.declare_dram_parameter("v", [BPC, TOK, D], BF16, isOutput=False)
    out = nc.declare_dram_parameter("out", [BPC, TOK, D], F32, isOutput=True)

    with tile.TileContext(nc) as tc:
        with (
            tc.tile_pool(name="xtp", bufs=2) as xtp,
            tc.tile_pool(name="vp", bufs=2) as vp,
            tc.tile_pool(name="ep", bufs=2) as ep,
            tc.tile_pool(name="statp", bufs=2) as statp,
            tc.tile_pool(name="op", bufs=4) as op,
            tc.tile_pool(name="pss", bufs=2, space="PSUM") as pss,
            tc.tile_pool(name="pso", bufs=4, space="PSUM") as pso,
        ):
            for b in range(BPC):
                # ---- load this block's data (next block overlaps, bufs=2)
                xts = []
                for d in range(DC):
                    t = xtp.tile([128, TOK], BF16, tag=f"xt{d}")
                    nc.sync.dma_start(out=t, in_=xt[b, d * 128:(d + 1) * 128, :])
                    xts.append(t)
                vts = []
                for a in range(TC):
                    t = vp.tile([128, D], BF16, tag=f"v{a}")
                    nc.sync.dma_start(out=t, in_=v[b, a * 128:(a + 1) * 128, :])
                    vts.append(t)

                # ---- S = Xd Xd^T (row-chunks), E = exp(S/32), row sums
                es = []
                recips = []
                for a in range(TC):
                    ps = pss.tile([128, TOK], F32, tag="ps_s")
                    for d in range(DC):
                        nc.tensor.matmul(
                            ps,
                            lhsT=xts[d][:, a * 128:(a + 1) * 128],
                            rhs=xts[d],
                            start=(d == 0),
                            stop=(d == DC - 1),
                        )
                    e = ep.tile([128, TOK], BF16, tag=f"e{a}")
                    sm = statp.tile([128, 1], F32, tag=f"sum{a}")
                    nc.scalar.activation(
                        out=e,
                        in_=ps,
                        func=mybir.ActivationFunctionType.Exp,
                        scale=SCALE,
                        accum_out=sm,
                    )
                    rc = statp.tile([128, 1], F32, tag=f"rc{a}")
                    nc.vector.reciprocal(rc, sm)
                    es.append(e)
                    recips.append(rc)

                # ---- O = E V, normalized on PSUM eviction.
                # E is symmetric: es[a][:, c-slice] == (E[c-chunk, a-chunk])^T,
                # exactly the [k, q] stationary operand the matmul needs.
                for c in range(TC):
                    for h in range(NH):
                        po = pso.tile([128, 512], F32, tag="ps_o")
                        for a in range(TC):
                            nc.tensor.matmul(
                                po,
                                lhsT=es[a][:, c * 128:(c + 1) * 128],
                                rhs=vts[a][:, h * 512:(h + 1) * 512],
                                start=(a == 0),
                                stop=(a == TC - 1),
                            )
                        ot = op.tile([128, 512], F32, tag="o")
                        # normalize while evicting PSUM (ACT: out = in * recip)
                        nc.scalar.mul(out=ot, in_=po, mul=recips[c])
                        nc.sync.dma_start(
                            out=out[b, c * 128:(c + 1) * 128, h * 512:(h + 1) * 512],
                            in_=ot,
                        )
    nc.compile()
    return nc


def _prepare_shards(x: np.ndarray):
    """Full x (4, 8192, 1024) f32 -> per-core bf16 shards in both layouts."""
    xd = x.reshape(B, NSEG, SEG, D)[:, :, ::DIL, :].reshape(NBLK, TOK, D)
    v_np = np.ascontiguousarray(xd).astype(ml_dtypes.bfloat16)
    # [blk, D, TOK] transposed layout for the QK^T contraction
    xt_np = np.ascontiguousarray(xd.transpose(0, 2, 1)).astype(ml_dtypes.bfloat16)
    in_maps = []
    for i in range(N_CORES):
        sl = slice(i * BPC, (i + 1) * BPC)
        in_maps.append(
            {
                "xt": np.ascontiguousarray(xt_np[sl]),
                "v": np.ascontiguousarray(v_np[sl]),
            }
        )
    return in_maps


def _run(x: np.ndarray, trace: bool = False):
    x = np.asarray(x, dtype=np.float32)
    assert x.shape == (B, S, D), x.shape
    nc = build_bass()
    in_maps = _prepare_shards(x)
    res = run_bass_kernel_spmd(nc, in_maps, list(range(N_CORES)), trace=trace)
    outs = [np.asarray(res.results[i]["out"], dtype=np.float32) for i in range(N_CORES)]
    full = np.concatenate(outs, axis=0)          # (32, 512, 1024)
    full = full.reshape(B, NSEG * TOK, D)        # (4, 4096, 1024)
    return full, res


def kernel(x: np.ndarray) -> np.ndarray:
    out, _ = _run(x, trace=False)
    return out


# revision 6
# speedup vs baseline: 1.1653x; 1.1653x over previous
"""v4: v3 + DMA-efficient layouts.

- Inputs are host-interleaved so each SBUF partition line is one 8 KB
  contiguous DRAM read (vs 1-2 KB in v1-v3; measured DMA efficiency was
  263 GB/s of 360 due to small packets): xt_h[b,p,dd,:] holds xT row
  dd*128+p, v_h[b,p,a,:] holds x row a*128+p. One or two DMAs per
  tensor per block instead of 12.
- Output rows assembled [128,1024] f32 per token-chunk -> natural-layout
  4 KB-line stores, 4 per block.
- Warmup matmuls accumulate into a `pso`-tagged psum slot (frees a PSUM
  bank), ps_s triple-buffered.
"""

import numpy as np
import ml_dtypes

import concourse.bass as bass
import concourse.bacc as bacc
import concourse.tile as tile
from concourse import mybir
from concourse.bass_utils import run_bass_kernel_spmd
from concourse.masks import make_identity

BF16 = mybir.dt.bfloat16
F32 = mybir.dt.float32

N_CORES = 8
B, S, D = 4, 8192, 1024
SEG = 1024
DIL = 2
TOK = SEG // DIL          # 512
NSEG = S // SEG           # 8
NBLK = B * NSEG           # 32
BPC = NBLK // N_CORES     # 4
TC = TOK // 128           # 4
DC = D // 128             # 8
NH = D // 512             # 2
SCALE = 1.0 / 32.0
N_WARMUP_MM = 16


def build_bass() -> bass.Bass:
    nc = bacc.Bacc()
    xt = nc.declare_dram_parameter("xt", [BPC, 128, DC, TOK], BF16, isOutput=False)
    v = nc.declare_dram_parameter("v", [BPC, 128, TC, D], BF16, isOutput=False)
    out = nc.declare_dram_parameter("out", [BPC, TOK, D], F32, isOutput=True)

    with tile.TileContext(nc) as tc:
        with (
            tc.tile_pool(name="const", bufs=1) as const,
            tc.tile_pool(name="xtp", bufs=2) as xtp,
            tc.tile_pool(name="vp", bufs=2) as vp,
            tc.tile_pool(name="ep", bufs=2) as ep,
            tc.tile_pool(name="statp", bufs=2) as statp,
            tc.tile_pool(name="op", bufs=3) as op,
            tc.tile_pool(name="pss", bufs=3, space="PSUM") as pss,
            tc.tile_pool(name="pst", bufs=2, space="PSUM") as pst,
            tc.tile_pool(name="pso", bufs=3, space="PSUM") as pso,
        ):
            ident = const.tile([128, 128], BF16)
            make_identity(nc, ident)

            # PE warm-up while preamble + first DMAs run (HAM un-throttle).
            warm = const.tile([128, TOK], BF16)
            nc.vector.memset(warm, 1.0)
            wps = pso.tile([128, TOK], F32, tag="ps_o", name="wps")
            for w in range(N_WARMUP_MM):
                nc.tensor.matmul(
                    wps,
                    lhsT=warm[:, 0:128],
                    rhs=warm,
                    start=(w == 0),
                    stop=(w == N_WARMUP_MM - 1),
                )

            for b in range(BPC):
                # ---- big-line loads: xt in two DMAs (4 d-chunks each), v in one
                xtb = xtp.tile([128, DC, TOK], BF16, tag="xtb")
                nc.sync.dma_start(out=xtb[:, : DC // 2, :], in_=xt[b, :, : DC // 2, :])
                nc.sync.dma_start(out=xtb[:, DC // 2 :, :], in_=xt[b, :, DC // 2 :, :])
                vb = vp.tile([128, TC, D], BF16, tag="vb")
                nc.sync.dma_start(out=vb, in_=v[b])

                es = [
                    ep.tile([128, TOK], BF16, tag=f"e{a}", name=f"e{a}")
                    for a in range(TC)
                ]

                # ---- upper-triangle scores + exp; mirror lower chunks
                for a in range(TC):
                    ncols = TOK - a * 128
                    ps = pss.tile([128, TOK], F32, tag="ps_s")
                    for d in range(DC):
                        nc.tensor.matmul(
                            ps[:, :ncols],
                            lhsT=xtb[:, d, a * 128:(a + 1) * 128],
                            rhs=xtb[:, d, a * 128:],
                            start=(d == 0),
                            stop=(d == DC - 1),
                        )
                    nc.scalar.activation(
                        out=es[a][:, a * 128:],
                        in_=ps[:, :ncols],
                        func=mybir.ActivationFunctionType.Exp,
                        scale=SCALE,
                    )
                    for c in range(a + 1, TC):
                        pt = pst.tile([128, 128], BF16, tag="ps_t")
                        nc.tensor.transpose(
                            pt, es[a][:, c * 128:(c + 1) * 128], ident
                        )
                        nc.vector.tensor_copy(
                            out=es[c][:, a * 128:(a + 1) * 128], in_=pt
                        )

                # ---- row sums & reciprocals (E symmetric: row sum == col sum)
                recips = []
                for a in range(TC):
                    sm = statp.tile([128, 1], F32, tag=f"sum{a}")
                    nc.vector.reduce_sum(out=sm, in_=es[a], axis=mybir.AxisListType.X)
                    rc = statp.tile([128, 1], F32, tag=f"rc{a}")
                    nc.vector.reciprocal(rc, sm)
                    recips.append(rc)

                # ---- O = E V; assemble full [128,1024] rows, one store per c
                for c in range(TC):
                    ot = op.tile([128, D], F32, tag="o")
                    for h in range(NH):
                        po = pso.tile([128, 512], F32, tag="ps_o")
                        for a in range(TC):
                            nc.tensor.matmul(
                                po,
                                lhsT=es[a][:, c * 128:(c + 1) * 128],
                                rhs=vb[:, a, h * 512:(h + 1) * 512],
                                start=(a == 0),
                                stop=(a == TC - 1),
                            )
                        if h == 0:
                            nc.vector.tensor_scalar_mul(
                                out=ot[:, h * 512:(h + 1) * 512],
                                in0=po,
                                scalar1=recips[c],
                            )
                        else:
                            nc.scalar.mul(
                                out=ot[:, h * 512:(h + 1) * 512],
                                in_=po,
                                mul=recips[c],
                            )
                    nc.sync.dma_start(
                        out=out[b, c * 128:(c + 1) * 128, :], in_=ot
                    )
    nc.compile()
    return nc


def _prepare_shards(x: np.ndarray):
    xd = x.reshape(B, NSEG, SEG, D)[:, :, ::DIL, :].reshape(NBLK, TOK, D)
    xd16 = xd.astype(ml_dtypes.bfloat16)
    # v_h[b, p, a, :] = x row a*128+p of block b   (8 KB partition lines)
    v_np = np.ascontiguousarray(
        xd16.reshape(NBLK, TC, 128, D).transpose(0, 2, 1, 3)
    )
    # xt_h[b, p, dd, :] = xT row dd*128+p of block b (8 KB partition lines)
    xt_np = np.ascontiguousarray(
        xd16.transpose(0, 2, 1).reshape(NBLK, DC, 128, TOK).transpose(0, 2, 1, 3)
    )
    in_maps = []
    for i in range(N_CORES):
        sl = slice(i * BPC, (i + 1) * BPC)
        in_maps.append(
            {
                "xt": np.ascontiguousarray(xt_np[sl]),
                "v": np.ascontiguousarray(v_np[sl]),
            }
        )
    return in_maps


def _run(x: np.ndarray, trace: bool = False):
    x = np.asarray(x, dtype=np.float32)
    assert x.shape == (B, S, D), x.shape
    nc = build_bass()
    in_maps = _prepare_shards(x)
    res = run_bass_kernel_spmd(nc, in_maps, list(range(N_CORES)), trace=trace)
    outs = [np.asarray(res.results[i]["out"], dtype=np.float32) for i in range(N_CORES)]
    full = np.concatenate(outs, axis=0)
    full = full.reshape(B, NSEG * TOK, D)
    return full, res


def kernel(x: np.ndarray) -> np.ndarray:
    out, _ = _run(x, trace=False)
    return out
